# revision 34
# baseline (speedup 1.0000x reference)
"""Deformable-correlation-fixed-weight kernel: 128-partition stream.

Work units (tap k, image row h), 864 total, processed as 7 batches of
128 units on all 128 partitions.  Batches 0..5 hold 4 taps x 32 rows
(25 pieces total; batch 6 = tap 8), so early batches touch only low
image rows and their bands load while the x/mean pipeline streams.
Each piece's 9-row impad band is loaded with its tap's (ky,kx) shift
folded into the DMA base, so every per-batch compute op is
tap-independent; the host pre-arranges offsets in the same piece
layout ([128, 7, 2, 96]) and the whole offset load is one flat DMA.
Each batch gates on its own band semaphore (DMA completions reorder
across engines, so shared counters race -- measured).

Numerics: clamp +-3.999 -> 9x10 hat window (col 9 identically zero),
fp16 coords/d-fields, bf16 window math; measured rel-err 0.0066 vs
the 2e-2 gate.  Mean image via PE matmuls over a full-size x buffer,
per-bank PSUM->SBUF copies on ScalarE with batch-0's hat evaluations
interleaved after copy 8 (the first prod must not wait for the whole
x-load-paced copy stream); staged mean writes (rows 0-37/38-69/70-95)
feed the band reads.  All DMAs issue from SyncE: GpSimd co-running
steals an SBUF port and slows every DVE op ~20% (measured).
"""

import numpy as np

B, C, H, W = 8, 128, 96, 96
K = 9
T = 9
HW = H * W
CLAMP = 3.9990234375
AWA = 9
AWI = 10
PAD = 5
PIMC = 106
PIMR = 107
NIMP = PIMR * PIMC  # 11342
BANDU = 9 * PIMC - PIMC + 953  # see below; actual value set explicitly
BANDU = 953                    # max in-band read offset 952, +1
NCH = 512
NCHUNK = HW // NCH  # 18
XRING = 8
ZCH = 710
NU = K * H          # 864 units
NB = (NU + 127) // 128  # 7 batches

# batch tiling: batches 0..5 hold 4 taps x 32 rows each, batch 6 = tap 8.
# Early batches only touch low image rows, so their bands load while the
# x/mean pipeline is still streaming.
_KS = [[0, 1, 2, 3], [4, 5, 6, 7]] * 3
_HB = [0, 0, 32, 32, 64, 64]
_PIECES = []  # (batch, p0, n, h0, k)
for _b in range(6):
    for _j in range(4):
        _PIECES.append((_b, 32 * _j, 32, _HB[_b], _KS[_b][_j]))
_PIECES.append((6, 0, 96, 0, 8))
# rowsk DMA stage per piece: which mean-write half must have landed
def _stage(piece):
    h0 = piece[3]
    return 1 if h0 == 0 and piece[0] < 6 else (2 if h0 == 32 else 3)
# Each batch waits for ALL pieces of its issue stage: DMA completions can
# reorder across engines, so counting only "this batch's" pieces races.
_CUM_ROWSK = [8, 8, 16, 16, 25, 25, 25]

_cached = {}


def _act_pos(b, which):
    off = {"absx": 1, "relux": 2, "absy": 3, "reluy": 4}[which]
    if b == 0:
        return 9 + off
    return 22 + 4 * (b - 1) + off


def _build_nc():
    import concourse.bass as bass
    import concourse.mybir as mybir
    from contextlib import ExitStack

    f32 = mybir.dt.float32
    f16 = mybir.dt.float16
    bf16 = mybir.dt.bfloat16
    Alu = mybir.AluOpType
    Act = mybir.ActivationFunctionType
    AX = mybir.AxisListType

    nc = bass.Bass(detect_race_conditions=False)

    x_ext = nc.declare_dram_parameter("x", [C, HW], f32, isOutput=False)
    off_ext = nc.declare_dram_parameter("offu", [128, NB * 2 * W], f32,
                                        isOutput=False)
    iota_ext = nc.declare_dram_parameter("iota19", [128, 19], f16,
                                         isOutput=False)
    ones_ext = nc.declare_dram_parameter("ones", [C, 1], f32, isOutput=False)
    out_ext = nc.declare_dram_parameter("out", [K, HW], f32, isOutput=True)

    impad = nc.dram_tensor("impad", [NIMP], bf16)

    with ExitStack() as ctx:
        x_ring = ctx.enter_context(nc.sbuf_tensor([C, HW], f32))
        ones_sb = ctx.enter_context(nc.sbuf_tensor([C, 1], f32))
        iota_sb = ctx.enter_context(nc.sbuf_tensor([128, 19], f16))
        off_sb = ctx.enter_context(nc.sbuf_tensor([128, NB, 2, W], f32))
        py_u = ctx.enter_context(nc.sbuf_tensor([128, NB, W], f16))
        px_u = ctx.enter_context(nc.sbuf_tensor([128, NB, W], f16))
        dX = ctx.enter_context(nc.sbuf_tensor([128, NB, W, AWI], f16))
        dY = ctx.enter_context(nc.sbuf_tensor([128, NB, W, AWA], f16))
        wX = ctx.enter_context(nc.sbuf_tensor([128, NB, W, AWI], bf16))
        wY = ctx.enter_context(nc.sbuf_tensor([128, NB, W, AWA], bf16))
        rowsk = ctx.enter_context(nc.sbuf_tensor([128, NB, BANDU], bf16))
        prod1 = ctx.enter_context(nc.sbuf_tensor([128, W, AWA, AWI], bf16))
        q1 = ctx.enter_context(nc.sbuf_tensor([128, W, AWA, 4], bf16))
        r1 = ctx.enter_context(nc.sbuf_tensor([128, W, AWA, 2], bf16))
        s0b = ctx.enter_context(nc.sbuf_tensor([128, W, AWA], bf16))
        s1 = ctx.enter_context(nc.sbuf_tensor([128, W, AWA], bf16))
        res = ctx.enter_context(nc.sbuf_tensor([128, NB, W], f32))
        m_flat = ctx.enter_context(nc.sbuf_tensor([1, HW], bf16))
        zt = ctx.enter_context(nc.sbuf_tensor([1, ZCH], bf16))
        psA = ctx.enter_context(nc.psum_tensor([1, 4096], f32))
        sB = ctx.enter_context(nc.semaphore("sB"))
        sC = ctx.enter_context(nc.semaphore("sC"))
        sRK = [ctx.enter_context(nc.semaphore(f"sR{b}"))
               for b in range(NB)]
        sO = ctx.enter_context(nc.semaphore("sO"))
        sX = ctx.enter_context(nc.semaphore("sX"))
        pe = ctx.enter_context(nc.semaphore("pe"))
        act = ctx.enter_context(nc.semaphore("act"))
        dve = ctx.enter_context(nc.semaphore("dve"))
        block = ctx.enter_context(nc.Block())

        def xchunk(sync, c):
            # 9 chunks of 1024 cols, full-size buffer: no ring gating, so
            # SyncE never blocks on PE progress before the mean writes
            sl = c * 1024
            sync.dma_start(
                out=x_ring[:, sl:sl + 1024],
                in_=x_ext[:, c * 1024:(c + 1) * 1024]).then_inc(sX, 16)

        @block.sync
        def _(sync):
            off_flat = bass.AP(
                tensor=off_sb[:].tensor, offset=off_sb[:].offset,
                ap=[list(off_sb[:].ap[0])] + [[1, NB * 2 * W]])
            sync.dma_start(out=off_flat, in_=off_ext[:]).then_inc(sB, 16)
            sync.dma_start(out=iota_sb[:], in_=iota_ext[:]).then_inc(sB, 16)
            sync.dma_start(out=ones_sb[:], in_=ones_ext[:]).then_inc(sB, 16)
            for c in range(4):
                xchunk(sync, c)
            sync.wait_ge(dve, 1)
            sync.dma_start(
                out=bass.AP(tensor=impad[:].tensor, offset=impad[:].offset,
                            ap=[[1, 1], [1, 5 * PIMC]]),
                in_=zt[:, 0:5 * PIMC]).then_inc(sC, 16)
            sync.dma_start(
                out=bass.AP(tensor=impad[:].tensor,
                            offset=impad[:].offset + 101 * PIMC,
                            ap=[[1, 1], [1, 6 * PIMC]]),
                in_=zt[:, 0:6 * PIMC]).then_inc(sC, 16)
            sync.dma_start(
                out=bass.AP(tensor=impad[:].tensor,
                            offset=impad[:].offset + 5 * PIMC,
                            ap=[[1, 1], [PIMC, H], [1, PAD]]),
                in_=zt[:, 0:H * PAD].rearrange("o (a b) -> o a b", a=H),
            ).then_inc(sC, 16)
            sync.dma_start(
                out=bass.AP(tensor=impad[:].tensor,
                            offset=impad[:].offset + 5 * PIMC + PAD + W,
                            ap=[[1, 1], [PIMC, H], [1, PAD]]),
                in_=zt[:, 0:H * PAD].rearrange("o (a b) -> o a b", a=H),
            ).then_inc(sC, 16)
            for c in range(4, 9):
                xchunk(sync, c)
            # staged mean writes: rows 0-37 / 38-69 / 70-95, each as soon
            # as its PSUM copies land; band pieces follow their stage
            def mwrite(r0, r1):
                sync.dma_start(
                    out=bass.AP(tensor=impad[:].tensor,
                                offset=impad[:].offset
                                + (PAD + r0) * PIMC + PAD,
                                ap=[[1, 1], [PIMC, r1 - r0], [1, W]]),
                    in_=m_flat[:, r0 * W:r1 * W].rearrange(
                        "o (r c) -> o r c", r=r1 - r0)).then_inc(sC, 16)

            def rowsk_piece(piece):
                b, p0, n, h0, k = piece
                ky, kx = k // 3, k % 3
                base = (h0 + ky) * PIMC + kx
                sync.dma_start(
                    out=rowsk[p0:p0 + n, b, :],
                    in_=bass.AP(tensor=impad[:].tensor,
                                offset=impad[:].offset + base,
                                ap=[[PIMC, n], [1, BANDU]])).then_inc(
                                    sRK[b], 16)

            sync.wait_ge(act, 8)
            mwrite(0, 38)
            sync.wait_ge(sC, 16 * 5)
            for piece in _PIECES:
                if _stage(piece) == 1:
                    rowsk_piece(piece)
            sync.wait_ge(act, 18)
            mwrite(38, 70)
            sync.wait_ge(sC, 16 * 6)
            for piece in _PIECES:
                if _stage(piece) == 2:
                    rowsk_piece(piece)
            sync.wait_ge(act, 22)
            mwrite(70, H)
            sync.wait_ge(sC, 16 * 7)
            for piece in _PIECES:
                if _stage(piece) == 3:
                    rowsk_piece(piece)
            for (b, p0, n, h0, k) in _PIECES:
                sync.wait_ge(dve, 4 + b)
                sync.dma_start(
                    out=bass.AP(tensor=out_ext[:].tensor,
                                offset=out_ext[:].offset + k * HW + h0 * W,
                                ap=[[W, n], [1, W]]),
                    in_=res[p0:p0 + n, b, :]).then_inc(sO, 16)

        @block.tensor
        def _(tensor):
            tensor.wait_ge(sB, 48)
            for g in range(NCHUNK):
                tensor.wait_ge(sX, 16 * (g // 2 + 1))
                if g >= 8:
                    c = g - 8
                    tensor.wait_ge(act, c + 1 if c <= 8 else c + 5)
                sl = g * NCH
                bk = (g % 8) * NCH
                nc.tensor.matmul(
                    psA[:, bk:bk + NCH],
                    ones_sb[:],
                    x_ring[:, sl:sl + NCH],
                    start=True, stop=True,
                ).then_inc(pe, 1)

        @block.scalar
        def _(scalar):
            def copy(g):
                scalar.wait_ge(pe, g + 1)
                bk = (g % 8) * NCH
                nc.scalar.activation(
                    m_flat[:, g * NCH:(g + 1) * NCH],
                    psA[:, bk:bk + NCH],
                    Act.Copy, scale=1.0 / C,
                ).then_inc(act, 1)
            def hats(b):
                if b == 0:
                    scalar.wait_ge(dve, 2)
                nc.scalar.activation(dX[:, b], dX[:, b],
                                     Act.Abs).then_inc(act, 1)
                nc.scalar.activation(wX[:, b], dX[:, b], Act.Relu,
                                     bias=1.0, scale=-1.0).then_inc(act, 1)
                if b == 0:
                    scalar.wait_ge(dve, 3)
                nc.scalar.activation(dY[:, b], dY[:, b],
                                     Act.Abs).then_inc(act, 1)
                nc.scalar.activation(wY[:, b], dY[:, b], Act.Relu,
                                     bias=1.0, scale=-1.0).then_inc(act, 1)

            for g in range(9):
                copy(g)
            hats(0)
            for g in range(9, NCHUNK):
                copy(g)
            for b in range(1, NB):
                hats(b)

        @block.vector
        def _(vector):
            nc.vector.memset(zt[:], 0.0).then_inc(dve, 1)
            vector.wait_ge(sB, 48)
            nc.vector.tensor_scalar(
                py_u[:], off_sb[:, :, 0, :],
                CLAMP, -CLAMP, Alu.min, Alu.max)
            nc.vector.tensor_scalar(
                px_u[:], off_sb[:, :, 1, :],
                CLAMP, -CLAMP, Alu.min, Alu.max)
            pxb = px_u[:].unsqueeze(3).broadcast_to([128, NB, W, AWI])
            iotX = (iota_sb[:, 0:AWI].unsqueeze(1).unsqueeze(1)
                    .broadcast_to([128, NB, W, AWI]))
            nc.vector.tensor_tensor(dX[:], pxb, iotX,
                                    Alu.subtract).then_inc(dve, 1)
            pyb = py_u[:].unsqueeze(3).broadcast_to([128, NB, W, AWA])
            iotY = (iota_sb[:, AWI:AWI + AWA].unsqueeze(1).unsqueeze(1)
                    .broadcast_to([128, NB, W, AWA]))
            nc.vector.tensor_tensor(dY[:], pyb, iotY,
                                    Alu.subtract).then_inc(dve, 1)
            for b in range(NB):
                vector.wait_ge(sRK[b], 16 * (4 if b < 6 else 1))
                vector.wait_ge(act, _act_pos(b, "relux"))
                wXb = wX[:, b].unsqueeze(2).broadcast_to([128, W, AWA, AWI])
                skb = bass.AP(
                    tensor=rowsk[:].tensor,
                    offset=rowsk[:].offset + b * BANDU,
                    ap=[list(rowsk[:].ap[0])]
                    + [[1, W], [PIMC, AWA], [1, AWI]])
                nc.vector.tensor_tensor(prod1[:], skb, wXb, Alu.mult)
                nc.vector.tensor_add(
                    q1[:], prod1[:, :, :, 0:4], prod1[:, :, :, 4:8])
                nc.vector.tensor_add(
                    r1[:], q1[:, :, :, 0:2], q1[:, :, :, 2:4])
                nc.vector.tensor_add(
                    s0b[:], r1[:, :, :, 0], r1[:, :, :, 1])
                nc.vector.tensor_add(s1[:], s0b[:], prod1[:, :, :, 8])
                vector.wait_ge(act, _act_pos(b, "reluy"))
                nc.vector.tensor_mul(s1[:], s1[:], wY[:, b])
                nc.vector.tensor_reduce(res[:, b, :], s1[:], AX.X,
                                        Alu.add).then_inc(dve, 1)

    return nc


def _get_nc():
    if "nc" not in _cached:
        _cached["nc"] = _build_nc()
    return _cached["nc"]


def _run(x, offset, trace=False):
    from concourse.bass_utils import run_bass_kernel_spmd

    nc = _get_nc()

    iota19 = np.tile(
        np.concatenate([np.arange(-4, 6), np.arange(-4, 5)]
                       ).astype(np.float16), (128, 1))
    ones = np.ones((C, 1), dtype=np.float32)

    in_maps = []
    for b_ in range(B):
        offb = offset[b_].reshape(2 * K, H, W)
        offu = np.zeros((128, NB, 2, W), dtype=np.float32)
        for (pb, p0, n, h0, k) in _PIECES:
            offu[p0:p0 + n, pb, 0, :] = offb[2 * k, h0:h0 + n, :]
            offu[p0:p0 + n, pb, 1, :] = offb[2 * k + 1, h0:h0 + n, :]
        in_maps.append({
            "x": np.ascontiguousarray(x[b_].reshape(C, HW), dtype=np.float32),
            "offu": np.ascontiguousarray(
                offu.reshape(128, NB * 2 * W)),
            "iota19": iota19,
            "ones": ones,
        })

    return run_bass_kernel_spmd(nc, in_maps, list(range(B)), trace=trace)


def kernel(x: np.ndarray, offset: np.ndarray, weight: np.ndarray) -> np.ndarray:
    results = _run(x, offset).results

    s = weight.reshape(C, T * K).sum(axis=0).astype(np.float32)  # [T*K]
    out = np.empty((B, T * K, H, W), dtype=np.float32)
    for b_ in range(B):
        samp = results[b_]["out"].reshape(K, H, W)
        for t in range(T):
            out[b_, t * K:(t + 1) * K] = (
                s[t * K:(t + 1) * K, None, None] * samp)
    return out
